# revision 27
# baseline (speedup 1.0000x reference)
"""KNN max-pooling kernel for Trainium2 (8 NeuronCores, SPMD).

out[m, :] = max_{s<16} feat[idx[m, s], :]   feat: [100000, 64] f32, idx: [100000, 16] i64

Strategy: shard the 100000 query rows across 8 cores (12500 each), replicate
the feature table in DRAM as fp16 (rel-err budget 2e-2 >> fp16's ~5e-4).
Each core fetches neighbors with the gpsimd SWDGE dma_gather. dma_gather
indices are int16 (max 32767) and elements must be multiples of 256B, so the
table is viewed as 25000 "quad" rows of 4x64 fp16 (512B): quad index = idx>>2
always fits. The wanted row inside each quad is selected with a
host-precomputed {0, -30000} additive mask (stored pair-duplicated so every
DVE operand keeps a stride-1 innermost dim -> fp16 fast path), then a
pairwise max tree on the vector engine collapses quads and the 16 neighbors.
Gather calls are capped at 1024 descriptors (larger calls hang real HW),
spread over all 4 SWDGE queues, with 4-deep tile buffering (deeper
buffering intermittently wedges the SWDGE ring-backpressure path on HW).
"""

import sys

if "/opt/trn_rl_repo" not in sys.path:
    sys.path.insert(0, "/opt/trn_rl_repo")

import numpy as np

import concourse.bacc as bacc
import concourse.tile as tile
from concourse import bass, mybir
from concourse.bass_utils import run_bass_kernel_spmd

# Problem shape (hardcoded per contract).
N_POINTS = 100000
N_QUERY = 100000
NSAMPLE = 16
C = 64

N_CORES = 8
M_LOC = N_QUERY // N_CORES   # 12500 queries per core
P = 128                      # queries per block (one per partition)
NB = 2                       # blocks per supertile
SUPER = P * NB               # queries per supertile
T = -(-M_LOC // SUPER)       # supertiles per core
M_PAD = T * SUPER

NQUAD = N_POINTS // 4        # 25000 quad rows (< 32768, int16-safe)
QC = 4 * C                   # 256 fp16 per quad row (512B)
NSLOT = NB * NSAMPLE         # gather slots per partition per supertile
NIDX = NSLOT * P             # gather descriptors per supertile
# Real-HW SWDGE rejects calls over 1024 descriptors (2048- and 1920-desc
# calls hang the device even though CoreSim accepts them): 8 slots per call.
GSPLIT = (8, 8, 8, 8)
NEG = -30000.0               # additive kill mask (fp16-exact, << min data)

_CACHE = {}


def _ap(base, offset, dims):
    """Free-dim view of a tile: partition dim from base, custom (stride, n)."""
    a = base[:]
    return bass.AP(a.tensor, offset, [a.ap[0]] + list(dims))


def _build_program():
    nc = bacc.Bacc("TRN2", target_bir_lowering=False, debug=False,
                   num_swdge_queues=4)
    feat_t = nc.dram_tensor("feat", [NQUAD, QC], mybir.dt.float16,
                            kind="ExternalInput")
    idx_t = nc.dram_tensor("idx_dev", [T, P, NIDX // 16], mybir.dt.int16,
                           kind="ExternalInput")
    mask_t = nc.dram_tensor("mask_dev", [T, P, NSLOT * 4, 2], mybir.dt.float16,
                            kind="ExternalInput")
    out_t = nc.dram_tensor("out_dev", [T, P, NB, C], mybir.dt.float16,
                           kind="ExternalOutput")

    with tile.TileContext(nc) as tc:
        with tc.tile_pool(name="big", bufs=4) as bigp, \
             tc.tile_pool(name="small", bufs=8) as smallp:
            for t in range(T):
                idx_tile = smallp.tile([P, NIDX // 16], mybir.dt.int16, tag="idx")
                nc.sync.dma_start(idx_tile[:], idx_t[t, :, :])
                mask_tile = smallp.tile([P, NSLOT * 4, 2], mybir.dt.float16,
                                        tag="mask")
                nc.sync.dma_start(mask_tile[:], mask_t[t, :, :, :])

                # stage[p, slot, :] = quad row for (query block*128+p, neighbor)
                # slot = b*16 + s; gather idx j = slot*128 + p
                stage = bigp.tile([P, NSLOT, QC], mybir.dt.float16, tag="stage")
                s0 = 0
                for g, ns in enumerate(GSPLIT):
                    nd = ns * P
                    nc.gpsimd.dma_gather(
                        out_ap=stage[:, s0:s0 + ns, :],
                        in_ap=feat_t[:],
                        idxs_ap=idx_tile[:, s0 * 8:(s0 + ns) * 8],
                        num_idxs=nd,
                        num_idxs_reg=nd,
                        elem_size=QC,
                        queue_num=(t * len(GSPLIT) + g) % 4,
                    )
                    s0 += ns

                # kill the 3 unwanted rows of each quad: stage += mask.
                # Views iterate (slot*4 rows, 32 c-pairs, 2): every operand
                # keeps a stride-1 innermost pair -> DVE fp16 fast path.
                st4 = _ap(stage, 0, [(C, NSLOT * 4), (2, C // 2), (1, 2)])
                mb4 = _ap(mask_tile, 0, [(2, NSLOT * 4), (0, C // 2), (1, 2)])
                nc.vector.tensor_tensor(out=st4, in0=st4, in1=mb4,
                                        op=mybir.AluOpType.add)

                # collapse quad: 4x64 -> 2x64 -> 1x64
                a01 = _ap(stage, 0, [(QC, NSLOT), (1, 2 * C)])
                a23 = _ap(stage, 2 * C, [(QC, NSLOT), (1, 2 * C)])
                nc.vector.tensor_tensor(out=a01, in0=a01, in1=a23,
                                        op=mybir.AluOpType.max)
                red = smallp.tile([P, NSLOT, C], mybir.dt.float16, tag="red")
                r0 = _ap(stage, 0, [(QC, NSLOT), (1, C)])
                r1 = _ap(stage, C, [(QC, NSLOT), (1, C)])
                nc.vector.tensor_tensor(out=red[:], in0=r0, in1=r1,
                                        op=mybir.AluOpType.max)

                # neighbor max: pairwise tree over the 16 slots of each block
                # (both blocks per instruction; all views stride-1 innermost).
                BS = NSAMPLE * C  # block stride in red
                for n in (8, 4, 2):
                    lo = _ap(red, 0, [(BS, NB), (C, n), (1, C)])
                    hi = _ap(red, n * C, [(BS, NB), (C, n), (1, C)])
                    nc.vector.tensor_tensor(out=lo, in0=lo, in1=hi,
                                            op=mybir.AluOpType.max)
                out_tile = smallp.tile([P, NB, C], mybir.dt.float16, tag="out")
                f0 = _ap(red, 0, [(BS, NB), (1, C)])
                f1 = _ap(red, C, [(BS, NB), (1, C)])
                ot = _ap(out_tile, 0, [(C, NB), (1, C)])
                nc.vector.tensor_tensor(out=ot, in0=f0, in1=f1,
                                        op=mybir.AluOpType.max)
                nc.sync.dma_start(out_t[t, :, :, :], out_tile[:])

    nc.compile()
    return nc


def _prep_inputs(idx):
    """idx [100000,16] -> per-core (idx_dev int16 wrapped, mask_dev fp16x2)."""
    idxq = (idx >> 2).astype(np.int16)
    rem = (idx & 3).astype(np.int64)
    idx_devs, mask_devs = [], []
    for k in range(N_CORES):
        q = np.zeros((M_PAD, NSAMPLE), np.int16)
        r = np.zeros((M_PAD, NSAMPLE), np.int64)
        q[:M_LOC] = idxq[k * M_LOC:(k + 1) * M_LOC]
        r[:M_LOC] = rem[k * M_LOC:(k + 1) * M_LOC]
        # flat gather index j = (b*16+s)*128 + p  ->  [T, NB, S, P]
        arr = q.reshape(T, NB, P, NSAMPLE).transpose(0, 1, 3, 2)
        flat = arr.reshape(T, NIDX)
        # idx j lives at (partition j%16, column j//16), replicated x8
        wrapped = flat.reshape(T, NIDX // 16, 16).transpose(0, 2, 1)
        idx_devs.append(np.ascontiguousarray(np.tile(wrapped, (1, 8, 1))))
        # mask[t, p, slot*4+k, pair] = 0 if k == rem else NEG (pair-doubled)
        rr = r.reshape(T, NB, P, NSAMPLE).transpose(0, 2, 1, 3).reshape(T, P, NSLOT)
        m = np.where(np.arange(4)[None, None, None, :] == rr[..., None],
                     np.float16(0.0), np.float16(NEG))
        m2 = np.repeat(m.reshape(T, P, NSLOT * 4, 1), 2, axis=3)
        mask_devs.append(np.ascontiguousarray(m2.astype(np.float16)))
    return idx_devs, mask_devs


def _unshard_out(outs):
    parts = []
    for o in outs:
        full = o.reshape(T, P, NB, C).transpose(0, 2, 1, 3).reshape(M_PAD, C)
        parts.append(full[:M_LOC].astype(np.float32))
    return np.concatenate(parts, axis=0)


def run(feat, idx, trace=False):
    if "nc" not in _CACHE:
        _CACHE["nc"] = _build_program()
    nc = _CACHE["nc"]

    featq = np.ascontiguousarray(
        feat.astype(np.float16).reshape(NQUAD, QC))
    idx_devs, mask_devs = _prep_inputs(idx)
    in_maps = [{"feat": featq, "idx_dev": idx_devs[k], "mask_dev": mask_devs[k]}
               for k in range(N_CORES)]

    res = run_bass_kernel_spmd(nc, in_maps, core_ids=list(range(N_CORES)),
                               trace=trace)
    out = _unshard_out([r["out_dev"] for r in res.results])
    return out, res.exec_time_ns


def kernel(feat, idx):
    out, _ = run(feat, idx, trace=False)
    return out
